# revision 39
# baseline (speedup 1.0000x reference)
"""Trainium2 Bass kernel for nn_DisCA (dual conv-block + channel attention).

Data-parallel over batch: 8 batch items -> 8 NeuronCores, one image per core.
Conv weights / BN affine replicated. BatchNorm batch statistics are obtained
with a per-block cross-core AllReduce of per-channel (sum, sumsq) ([1,1024]
f32 each). The BN affine transform is folded algebraically into the
attention-score matrix,
    scores = a1[c]*a2[d]*S[c,d] + (a1*r1)[c]*b2bn[d] + b1bn[c]*(a2*r2+N*b2bn)[d]
where S is the raw (pre-BN) Gram matrix and r_i are local per-channel row
sums, so the raw score matmul overlaps the AllReduce latency.

v4 schedule (vs v2):
  - block order swapped: conv_block(x2) runs FIRST, so its AllReduce (which
    feeds the expensive column/d-axis broadcast affine chain) is fully
    hidden under conv_block(x1); the second AllReduce (x1/row side) only
    gates a cheap per-partition [128,4] chain.
  - the x2-side affine params are computed in broadcast [128,512] layout
    directly from row-broadcast matmuls of the AllReduced sums (no
    per-partition->row->broadcast conversion pipeline).
  - no keep-warm filler matmuls (measured: they burned ~54us of PE and did
    not keep HAM warm); the AR window is filled with the Gram + the x2
    affine chain + the hoisted GpSimd score scalings.
  - ACT tables: only Identity+Lrelu preloaded at head; Sqrt loads with the
    (hidden) x2 chain, Exp via a dummy in the same window.
  - conv2's Square runs on the DVE instead of ACT.
"""

import os
import sys

for _p in ("/opt/trn_rl_repo", "/root/.axon_site/_ro/trn_rl_repo"):
    if os.path.isdir(_p) and _p not in sys.path:
        sys.path.insert(0, _p)

import numpy as np

import concourse.bacc as bacc
import concourse.mybir as mybir
from concourse.tile import TileContext, add_dep_helper
from concourse.bass_utils import run_bass_kernel_spmd
from concourse.masks import make_identity

F32 = mybir.dt.float32
F32R = mybir.dt.float32r
F16 = mybir.dt.float16


def _r(ap):
    """Reinterpret an fp32 AP as float32r (single-pass full-rate PE mode)."""
    return ap.bitcast(F32R)


AF = mybir.ActivationFunctionType
ALU = mybir.AluOpType

NCORES = 8
B, C, H, W = 8, 512, 32, 32
N = H * W                      # 1024 spatial positions per image
CMID = 256                     # conv1 output channels
HP = H + 2                     # padded spatial dim (34)
BN_EPS = 1e-5
LRELU_SLOPE = 0.01
M_TOTAL = float(B * N)         # BN statistic count (full batch)

KC = C // 128                  # 4 channel chunks of x
KM = CMID // 128               # 2 channel chunks of mid features


def build_kernel():
    nc = bacc.Bacc("TRN2", target_bir_lowering=False, debug=False,
                   num_devices=NCORES)

    # ---- DRAM I/O -------------------------------------------------------
    x1d = nc.dram_tensor("x1s", [128, 4096], F32, kind="ExternalInput")
    x2d = nc.dram_tensor("x2s", [128, 4096], F32, kind="ExternalInput")
    xd = nc.dram_tensor("xs", [128, 4096], F32, kind="ExternalInput")
    w1d = nc.dram_tensor("w1t", [128, 1024], F32, kind="ExternalInput")
    w2d = nc.dram_tensor("w2t", [128, 9216], F32, kind="ExternalInput")
    # vecs rows: 0=b2, 1=gamma, 2=bn_bias, 3=beta(col0), 4=b1(cols 0:256)
    vecd = nc.dram_tensor("vecs", [8, 512], F32, kind="ExternalInput")
    outd = nc.dram_tensor("out", [128, 4096], F32, kind="ExternalOutput")

    # cc slots: 0 = x2 block (r2|q2), 1 = x1 block (r1|q1).  fp16 on the
    # wire halves the mesh-AR payload (latency scales ~4.5us/KB); the
    # value ranges (|sum| < 2e4) fit fp16 and the affine algebra keeps the
    # large a2*r2loc / N*b2bn cancellation in LOCAL f32 quantities, so the
    # added score perturbation is only ~0.02.
    cc_in = nc.dram_tensor("cc_in", [1, 2048], F16, kind="Internal")
    cc_out = nc.dram_tensor("cc_out", [1, 2048], F16, kind="Internal",
                            addr_space="Shared")
    cw_in = nc.dram_tensor("cw_in", [1, 8], F32, kind="Internal")
    cw_out = nc.dram_tensor("cw_out", [1, 8], F32, kind="Internal",
                            addr_space="Shared")

    with TileContext(nc, num_cores=NCORES) as tc:
        with (
            tc.tile_pool(name="const", bufs=1) as const,
            tc.tile_pool(name="big", bufs=1) as big,
            tc.tile_pool(name="work", bufs=2) as work,
            tc.tile_pool(name="vec", bufs=1) as vec,
            tc.tile_pool(name="ps", bufs=6, space="PSUM") as ps,
            tc.tile_pool(name="psS", bufs=2, space="PSUM") as psS,
        ):
            # ---- input DMAs first: conv1(x2)'s operands lead -----------
            # tiny b2 row first (an early PE matmul needs it), then w1 and
            # all of x2 back-to-back: conv1(x2) n2=0 starts after the first
            # 4 half-chunks and n2=1 follows without a DMA gap
            b2row = const.tile([1, 512], F32)
            nc.sync.dma_start(out=_r(b2row[:]), in_=_r(vecd[0:1, :]))
            w1t = big.tile([128, 1024], F32)
            nc.sync.dma_start(out=_r(w1t[:]), in_=_r(w1d[:]))
            x2s = work.tile([128, 4096], F32, tag="xin")
            for k in range(KC):
                nc.sync.dma_start(out=_r(x2s[:, 1024 * k:1024 * k + 512]),
                                  in_=_r(x2d[:, 1024 * k:1024 * k + 512]))
            for k in range(KC):
                nc.sync.dma_start(
                    out=_r(x2s[:, 1024 * k + 512:1024 * (k + 1)]),
                    in_=_r(x2d[:, 1024 * k + 512:1024 * (k + 1)]))

            # ---- constants / small tiles -------------------------------
            identity = const.tile([128, 128], F32)
            make_identity(nc, identity)
            ones_col = const.tile([128, 1], F32)   # lhsT for partition sums
            nc.vector.memset(ones_col[:], 1.0)
            nc.scalar.copy(_r(ones_col[:]), ones_col[:])
            ones_row = const.tile([1, 128], F32)   # K=1 lhsT (f32 mms)
            nc.vector.memset(ones_row[:], 1.0)
            onesm_row = const.tile([1, 128], F32)  # K=1 lhsT, value 1/M
            nc.vector.memset(onesm_row[:], 1.0 / M_TOTAL)
            one_one = const.tile([1, 1], F32)      # rhs for transpose-in mms
            nc.vector.memset(one_one[:], 1.0)
            one_one16 = const.tile([1, 1], F16)    # f16 rhs for fold mms
            nc.vector.memset(one_one16[:], 1.0)
            onesm_row16 = const.tile([1, 128], F16)  # f16 K=1 lhsT, 1/M
            nc.vector.memset(onesm_row16[:], 1.0 / M_TOTAL)
            eps_pp = const.tile([128, 1], F32)     # BN eps as bias AP
            nc.vector.memset(eps_pp[:], BN_EPS)
            zrow = const.tile([128, 32], F32)      # zero source for borders
            nc.vector.memset(zrow[:], 0.0)
            zrowf = const.tile([1, 8], F32)
            nc.vector.memset(zrowf[:], 0.0)
            # preload only the conv-phase ACT tables; Sqrt/Exp load in the
            # (hidden) post-conv window
            tld = const.tile([1, 8], F32)
            nc.scalar.activation(tld[:], zrowf[:], AF.Identity)
            nc.scalar.activation(tld[:], zrowf[:], AF.Lrelu,
                                 alpha=LRELU_SLOPE)

            b1pp = const.tile([128, KM], F32)      # b1 per-partition chunks
            for m in range(KM):
                nc.sync.dma_start(out=b1pp[:, m:m + 1],
                                  in_=vecd[4:5, 128 * m:128 * (m + 1)])
            # small affine-vector DMAs BEFORE the big w2t so the head
            # broadcast matmuls don't stall the conv1 PE queue
            gb = const.tile([1, 1024], F32)        # gamma | bn_bias rows
            nc.sync.dma_start(out=gb[0:1, 0:512], in_=vecd[1:2, :])
            nc.sync.dma_start(out=gb[0:1, 512:1024], in_=vecd[2:3, :])
            betar = const.tile([1, 1], F32)
            nc.sync.dma_start(out=betar[:], in_=vecd[3:4, 0:1])
            gpp = const.tile([128, KC], F32)
            nc.sync.dma_start(
                out=gpp[:],
                in_=vecd[1:2, :].rearrange("o (j p) -> o p j", p=128))
            bnbpp = const.tile([128, KC], F32)
            nc.sync.dma_start(
                out=bnbpp[:],
                in_=vecd[2:3, :].rearrange("o (j p) -> o p j", p=128))
            w2t = big.tile([128, 9216], F32)
            for j in range(3):
                nc.sync.dma_start(out=_r(w2t[:, 3072 * j:3072 * (j + 1)]),
                                  in_=_r(w2d[:, 3072 * j:3072 * (j + 1)]))

            # b2 broadcast [128,512]: preloaded into each conv2 PSUM acc
            b2bps = ps.tile([128, 512], F32, tag="ps")
            nc.tensor.matmul(b2bps[:], ones_row[:], b2row[:],
                             start=True, stop=True)
            b2bcast = const.tile([128, 512], F32)
            nc.vector.tensor_copy(b2bcast[:], b2bps[:])

            # tiny warmup AllReduce: pays the ~50-70us first-collective
            # channel-init cost while the input DMAs stream, so the real
            # stats ARs hit a warm path
            warm = const.tile([1, 8], F32)
            nc.vector.memset(warm[:], 1.0)
            nc.sync.dma_start(out=cw_in[:], in_=warm[:])
            nc.gpsimd.collective_compute(
                "AllReduce", ALU.add,
                replica_groups=[list(range(NCORES))],
                ins=[cw_in[:]], outs=[cw_out[:]])

            x1s = work.tile([128, 4096], F32, tag="xin")
            for j in range(2):
                nc.sync.dma_start(out=_r(x1s[:, 2048 * j:2048 * (j + 1)]),
                                  in_=_r(x1d[:, 2048 * j:2048 * (j + 1)]))

            gammabc = const.tile([128, 512], F32)
            bnbbc = const.tile([128, 512], F32)
            betapp = vec.tile([128, 1], F32)

            # conv1 output, 3 horizontally-pre-shifted copies (kw = 0,1,2),
            # each vertically zero-padded to 34 rows of 32 contiguous cols.
            NROW = HP * W                       # 1088 elems per copy/chunk
            y1c = big.tile([128, 3 * KM * NROW], F32)

            def y1base(kw, k):
                return (kw * KM + k) * NROW

            def ycv(kw, k):
                return y1c[:, y1base(kw, k):y1base(kw, k) + NROW].rearrange(
                    "p (r c) -> p r c", c=W)
            zr = zrow[:].rearrange("p (a c) -> p a c", a=1)      # [128,1,32]
            zc = zrow[:].rearrange("p (c a) -> p c a", a=1)      # [128,32,1]
            for kw in range(3):
                for k in range(KM):
                    v = ycv(kw, k)
                    nc.scalar.copy(_r(v[:, 0:1, :]), zr)         # top row
                    nc.scalar.copy(_r(v[:, HP - 1:HP, :]), zr)   # bottom
                    if kw == 0:
                        nc.scalar.copy(_r(v[:, 1:HP - 1, 0:1]), zc)
                    if kw == 2:
                        nc.scalar.copy(_r(v[:, 1:HP - 1, W - 1:W]), zc)

            f1t = big.tile([128, 4096], F32)
            f2t = big.tile([128, 4096], F32)
            stats = const.tile([1, 2048], F32)     # local r2|q2 , r1|q1
            stats16 = const.tile([1, 2048], F16)   # f16 wire copies

            # ---- one conv block: x -> conv1 -> pad -> conv2 -> lrelu ---
            def conv_block(xin, ft, si):
                for n2 in range(2):
                    for m in range(KM):
                        acc = ps.tile([128, 512], F32, tag="ps")
                        for k in range(KC):
                            nc.tensor.matmul(
                                acc[:],
                                _r(w1t[:, 256 * k + 128 * m:256 * k + 128 * (m + 1)]),
                                _r(xin[:, 1024 * k + 512 * n2:1024 * k + 512 * (n2 + 1)]),
                                start=(k == 0), stop=(k == KC - 1))
                        accv = acc[:].rearrange("p (r c) -> p r c", c=W)
                        row0 = (1 + 16 * n2) * W
                        nc.scalar.activation(
                            _r(y1c[:, y1base(1, m) + row0:y1base(1, m) + row0 + 512]),
                            acc[:], AF.Identity, bias=b1pp[:, m:m + 1])
                        d0 = ycv(0, m)
                        nc.scalar.activation(
                            _r(d0[:, 1 + 16 * n2:17 + 16 * n2, 1:32]),
                            accv[:, :, 0:31], AF.Identity,
                            bias=b1pp[:, m:m + 1])
                        d2 = ycv(2, m)
                        nc.scalar.activation(
                            _r(d2[:, 1 + 16 * n2:17 + 16 * n2, 0:31]),
                            accv[:, :, 1:32], AF.Identity,
                            bias=b1pp[:, m:m + 1])

                # conv2 (3x3) -> transposed output F^T[n, c], s-outer
                racc = psS.tile([1, 512], F32, tag="stat")
                qacc = psS.tile([1, 512], F32, tag="stat")
                for s in range(8):
                    acc = ps.tile([128, 512], F32, tag="ps")
                    nc.vector.tensor_copy(acc[:], b2bcast[:])
                    for kh in range(3):
                        for kw in range(3):
                            t = kh * 3 + kw
                            for k in range(KM):
                                off = y1base(kw, k) + (4 * s + kh) * W
                                rhs = w2t[:, (2 * t + k) * 512:(2 * t + k + 1) * 512]
                                last = (kh == 2 and kw == 2 and k == KM - 1)
                                nc.tensor.matmul(acc[:],
                                                 _r(y1c[:, off:off + 128]),
                                                 _r(rhs),
                                                 start=False, stop=last,
                                                 skip_group_check=True)
                    nc.scalar.activation(_r(ft[:, 512 * s:512 * (s + 1)]),
                                         acc[:], AF.Lrelu,
                                         alpha=LRELU_SLOPE)
                    sq = work.tile([128, 512], F32, tag="sq")
                    nc.vector.tensor_mul(_r(sq[:]),
                                         ft[:, 512 * s:512 * (s + 1)],
                                         ft[:, 512 * s:512 * (s + 1)])
                    nc.tensor.matmul(racc[:], _r(ones_col[:]),
                                     _r(ft[:, 512 * s:512 * (s + 1)]),
                                     start=(s == 0), stop=(s == 7))
                    nc.tensor.matmul(qacc[:], _r(ones_col[:]), _r(sq[:]),
                                     start=(s == 0), stop=(s == 7))

                # local stats -> sbuf (f32 for local use, f16 for the wire)
                # + AllReduce for this block
                nc.vector.tensor_copy(
                    stats16[0:1, 1024 * si:1024 * si + 512], racc[:])
                nc.vector.tensor_copy(
                    stats16[0:1, 1024 * si + 512:1024 * (si + 1)], qacc[:])
                nc.sync.dma_start(out=cc_in[0:1, 1024 * si:1024 * (si + 1)],
                                  in_=stats16[0:1, 1024 * si:1024 * (si + 1)])
                nc.vector.tensor_copy(stats[0:1, 1024 * si:1024 * si + 512],
                                      racc[:])
                nc.vector.tensor_copy(
                    stats[0:1, 1024 * si + 512:1024 * (si + 1)], qacc[:])
                nc.gpsimd.collective_compute(
                    "AllReduce", ALU.add,
                    replica_groups=[list(range(NCORES))],
                    ins=[cc_in[0:1, 1024 * si:1024 * (si + 1)]],
                    outs=[cc_out[0:1, 1024 * si:1024 * (si + 1)]])

            def tail(xs):
                # ---- x2-side (column/d) affine in broadcast layout ------
                # AR slot 0 landed while conv_block(x1) ran (trigger ~50us
                # earlier), so these matmuls lead the tail PE queue and the
                # DVE chain runs concurrently with the Gram below.
                g2row = vec.tile([1, 1024], F16)
                nc.sync.dma_start(out=g2row[:], in_=cc_out[0:1, 0:1024])
                pm2 = ps.tile([128, 512], F32, tag="ps")
                nc.tensor.matmul(pm2[:], onesm_row16[:], g2row[0:1, 0:512],
                                 start=True, stop=True)   # mean2 bcast
                pq2 = ps.tile([128, 512], F32, tag="ps")
                nc.tensor.matmul(pq2[:], onesm_row16[:],
                                 g2row[0:1, 512:1024],
                                 start=True, stop=True)   # E[x2^2] bcast
                # local r2 row (stats slot 0) -> broadcast
                pr2l = ps.tile([128, 512], F32, tag="ps")
                nc.tensor.matmul(pr2l[:], ones_row[:], stats[0:1, 0:512],
                                 start=True, stop=True)
                r2locbc = vec.tile([128, 512], F32)
                nc.vector.tensor_copy(r2locbc[:], pr2l[:])

                # ---- raw Gram matmuls (no AR dependency) ----------------
                ssb = big.tile([128, 2048], F32)
                for m in range(KC):
                    sacc = ps.tile([128, 512], F32, tag="ps")
                    for k in range(8):
                        nc.tensor.matmul(
                            sacc[:],
                            _r(f1t[:, 512 * k + 128 * m:512 * k + 128 * (m + 1)]),
                            _r(f2t[:, 512 * k:512 * (k + 1)]),
                            start=(k == 0), stop=(k == 7))
                    nc.vector.tensor_copy(ssb[:, 512 * m:512 * (m + 1)],
                                          sacc[:])

                # x2-side chain on DVE (concurrent with the Gram); temps
                # rotate through a 2-slot work tag to save SBUF
                m2s = vec.tile([128, 512], F32)
                nc.vector.tensor_copy(m2s[:], pm2[:])
                t1b = work.tile([128, 512], F32, tag="chain")
                nc.vector.tensor_mul(t1b[:], m2s[:], m2s[:])
                varb = work.tile([128, 512], F32, tag="chain")
                nc.vector.tensor_sub(varb[:], pq2[:], t1b[:])
                sd2 = work.tile([128, 512], F32, tag="chain")
                nc.scalar.activation(sd2[:], varb[:], AF.Sqrt,
                                     bias=eps_pp[:])
                a2b = vec.tile([128, 512], F32)
                # ~5x faster than reciprocal(); 18-bit accuracy is ample
                # for the BN scale (sd2 in [~0.1, ~2], no edge cases)
                nc.vector.reciprocal_approx_fast(a2b[:], sd2[:])
                nc.vector.tensor_mul(a2b[:], a2b[:], gammabc[:])   # a2
                t2b = work.tile([128, 512], F32, tag="chain")
                nc.vector.tensor_mul(t2b[:], m2s[:], a2b[:])
                b2bc = vec.tile([128, 512], F32)
                nc.vector.tensor_sub(b2bc[:], bnbbc[:], t2b[:])    # b2bn
                t3b = work.tile([128, 512], F32, tag="chain")
                nc.vector.tensor_mul(t3b[:], a2b[:], r2locbc[:])
                wbc = vec.tile([128, 512], F32)
                nc.vector.scalar_tensor_tensor(
                    wbc[:], b2bc[:], float(N), t3b[:],
                    op0=ALU.mult, op1=ALU.add)                     # w

                # score column-scalings on the DVE (NOT GpSimd: the GpSimd
                # queue blocks inside collective_compute's completion wait,
                # so GpSimd work issued after an AR cannot precede it);
                # m=0,1 hoisted into the AR window, m=2,3 in the m-loop
                tmuls = []
                for m in range(2):
                    tmul = work.tile([128, 512], F32, tag="tmul", bufs=2)
                    nc.vector.tensor_mul(tmul[:],
                                         ssb[:, 512 * m:512 * (m + 1)],
                                         a2b[:])
                    tmuls.append(tmul)

                # local r1 (stats slot 1) into per-partition layout
                r1lps = ps.tile([128, KC], F32, tag="ps")
                for j in range(KC):
                    nc.tensor.matmul(r1lps[:, j:j + 1],
                                     stats[0:1, 1024 + 128 * j:1024 + 128 * (j + 1)],
                                     one_one[:], start=True, stop=True)
                rlpp = vec.tile([128, KC], F32)
                nc.vector.tensor_copy(rlpp[:], r1lps[:])

                # scheduler fence: nothing below may hoist above this point
                tc.no_sync_barrier()

                # ---- x1-side (row/c) affine params, per-partition -------
                # (these wait on AR slot 1 -- the only exposed latency)
                # contiguous row DMA + 8 transpose-in fold matmuls beats
                # the old per-element scatter DMA by several us
                g1row = vec.tile([1, 1024], F16)
                nc.sync.dma_start(out=g1row[:], in_=cc_out[0:1, 1024:2048])
                ppb = ps.tile([128, 2 * KC], F32, tag="ps")
                # q (sumsq) folds first so the variance-path DVE ops start
                # while the r folds are still on the PE
                for j in range(KC, 2 * KC):
                    nc.tensor.matmul(ppb[:, j:j + 1],
                                     g1row[0:1, 128 * j:128 * (j + 1)],
                                     one_one16[:], start=True, stop=True)
                s1pp = vec.tile([128, KC], F32)
                nc.vector.tensor_copy(s1pp[:], ppb[:, KC:2 * KC])
                nc.vector.tensor_scalar(s1pp[:], s1pp[:], 1.0 / M_TOTAL,
                                        BN_EPS, op0=ALU.mult, op1=ALU.add)
                for j in range(KC):
                    nc.tensor.matmul(ppb[:, j:j + 1],
                                     g1row[0:1, 128 * j:128 * (j + 1)],
                                     one_one16[:], start=True, stop=True)
                r1pp = vec.tile([128, KC], F32)
                nc.vector.tensor_copy(r1pp[:], ppb[:, 0:KC])
                a1pp = vec.tile([128, KC], F32)
                nc.vector.tensor_scalar_mul(r1pp[:], r1pp[:], 1.0 / M_TOTAL)
                nc.vector.tensor_mul(a1pp[:], r1pp[:], r1pp[:])    # mean^2
                nc.vector.tensor_sub(s1pp[:], s1pp[:], a1pp[:])    # var+eps
                sd1 = vec.tile([128, KC], F32)
                nc.scalar.activation(sd1[:], s1pp[:], AF.Sqrt)
                # prefetch the Exp table right AFTER the last Sqrt: the
                # dummy READS sd1 so the scheduler cannot hoist it above
                # the Sqrt (which would re-thrash the 2-entry table cache),
                # and the DMA reader keeps it from being dead-code
                # eliminated.  Its ~1.3us load hides under the DVE chain.
                tld2 = vec.tile([1, KC], F32)
                nc.scalar.activation(tld2[:], sd1[0:1, :], AF.Exp)
                nc.sync.dma_start(out=cw_in[0:1, 0:KC], in_=tld2[:])
                nc.vector.reciprocal(s1pp[:], sd1[:])
                nc.vector.tensor_mul(a1pp[:], gpp[:], s1pp[:])     # a1
                b1bnpp = vec.tile([128, KC], F32)
                nc.vector.tensor_mul(b1bnpp[:], r1pp[:], a1pp[:])
                nc.vector.tensor_sub(b1bnpp[:], bnbpp[:], b1bnpp[:])
                u_pp = vec.tile([128, KC], F32)
                nc.vector.tensor_mul(u_pp[:], a1pp[:], rlpp[:])

                # ---- scores + softmax-exp + transpose + apply ----------
                et = y1c                 # E^T overlaid on dead y1c
                scvec = vec.tile([128, KC], F32)
                for m in range(KC):
                    if m >= 2:      # refill the 2-buf tmul rotation.  On
                        # GpSimd: it is idle once the last AR's completion
                        # wait clears, and this keeps the 0.7us op off the
                        # DVE, which is the m-loop's co-critical engine.
                        tmul = work.tile([128, 512], F32, tag="tmul",
                                         bufs=2)
                        nc.gpsimd.tensor_mul(tmul[:],
                                             ssb[:, 512 * m:512 * (m + 1)],
                                             a2b[:])
                        tmuls.append(tmul)
                    rk = work.tile([128, 512], F32, tag="rk")
                    nc.vector.tensor_scalar(rk[:], b2bc[:], u_pp[:, m:m + 1],
                                            None, op0=ALU.mult)
                    nc.vector.scalar_tensor_tensor(
                        rk[:], wbc[:], b1bnpp[:, m:m + 1], rk[:],
                        op0=ALU.mult, op1=ALU.add)
                    sc = work.tile([128, 512], F32, tag="scores")
                    nc.vector.scalar_tensor_tensor(
                        sc[:], tmuls[m][:], a1pp[:, m:m + 1], rk[:],
                        op0=ALU.mult, op1=ALU.add)
                    nmx = vec.tile([128, 1], F32, tag="nmx")
                    nc.vector.tensor_reduce(nmx[:], sc[:],
                                            axis=mybir.AxisListType.X,
                                            op=ALU.max, negate=True)
                    esum = vec.tile([128, 1], F32, tag="esum")
                    ee = work.tile([128, 512], F32, tag="ee")
                    nc.scalar.activation(ee[:], sc[:], AF.Exp, bias=nmx[:],
                                         accum_out=esum[:])
                    nc.vector.reciprocal(esum[:], esum[:])
                    nc.vector.tensor_mul(scvec[:, m:m + 1], esum[:],
                                         betapp[:])
                    for j in range(KC):
                        tp = ps.tile([128, 128], F32, tag="ps")
                        nc.tensor.transpose(tp[:],
                                            ee[:, 128 * j:128 * (j + 1)],
                                            identity[:])
                        nc.scalar.copy(
                            _r(et[:, 512 * j + 128 * m:512 * j + 128 * (m + 1)]),
                            tp[:])
                    for n2 in range(2):
                        oacc = ps.tile([128, 512], F32, tag="ps")
                        for k in range(KC):
                            nc.tensor.matmul(
                                oacc[:],
                                _r(et[:, 512 * k + 128 * m:512 * k + 128 * (m + 1)]),
                                _r(xs[:, 1024 * k + 512 * n2:1024 * k + 512 * (n2 + 1)]),
                                start=(k == 0), stop=(k == KC - 1))
                        ot = work.tile([128, 512], F32, tag="ot")
                        nc.scalar.mul(ot[:], oacc[:], scvec[:, m:m + 1])
                        nc.sync.dma_start(
                            out=outd[:, 1024 * m + 512 * n2:1024 * m + 512 * (n2 + 1)],
                            in_=ot[:])

            conv_block(x2s, f2t, 0)
            # gamma / bn_bias / beta broadcasts: issued at the block
            # transition (NOT the head: there they sit in the PE queue
            # ahead of conv1 waiting on the late gb DMA, delaying the
            # first conv matmul by ~4us)
            for (dst, src) in ((gammabc, gb[0:1, 0:512]),
                               (bnbbc, gb[0:1, 512:1024])):
                bcp = ps.tile([128, 512], F32, tag="ps")
                nc.tensor.matmul(bcp[:], ones_row[:], src,
                                 start=True, stop=True)
                nc.vector.tensor_copy(dst[:], bcp[:])
            bps = ps.tile([128, 1], F32, tag="ps")
            nc.tensor.matmul(bps[:], ones_row[:], betar[:],
                             start=True, stop=True)
            nc.vector.tensor_copy(betapp[:], bps[:])
            # x reuses x2's slot (x2 is dead after its conv1)
            xs = work.tile([128, 4096], F32, tag="xin")
            for j in range(2):
                nc.sync.dma_start(out=_r(xs[:, 2048 * j:2048 * (j + 1)]),
                                  in_=_r(xd[:, 2048 * j:2048 * (j + 1)]))
            conv_block(x1s, f1t, 1)
            tail(xs)

    nc.compile()
    return nc


_NC_CACHE = []


def _get_nc():
    if not _NC_CACHE:
        _NC_CACHE.append(build_kernel())
    return _NC_CACHE[0]


def _prep_shared(w1, b1, w2, b2, gamma, bn_bias, beta):
    w1m = w1.reshape(CMID, C).astype(np.float32)
    w1t = np.ascontiguousarray(
        w1m.T.reshape(KC, 128, CMID).transpose(1, 0, 2).reshape(128, KC * CMID))
    w2t = np.empty((128, 9216), dtype=np.float32)
    for kh in range(3):
        for kw in range(3):
            t = kh * 3 + kw
            wt = w2[:, :, kh, kw].T  # [256 in, 512 out]
            for k in range(KM):
                w2t[:, (2 * t + k) * 512:(2 * t + k + 1) * 512] = \
                    wt[128 * k:128 * (k + 1), :]
    vecs = np.zeros((8, 512), dtype=np.float32)
    vecs[0] = b2
    vecs[1] = gamma
    vecs[2] = bn_bias
    vecs[3, 0] = np.asarray(beta).reshape(-1)[0]
    vecs[4, :CMID] = b1
    return w1t, w2t, vecs


def _chunk_img(img):
    # [512, 1024] -> [128, 4096] with channel chunk k at cols 1024k
    return np.ascontiguousarray(
        img.reshape(KC, 128, N).transpose(1, 0, 2).reshape(128, KC * N))


def kernel(x, x1, x2, w1, b1, w2, b2, gamma, bn_bias, beta, **run_kw):
    nc = _get_nc()
    w1t, w2t, vecs = _prep_shared(w1, b1, w2, b2, gamma, bn_bias, beta)
    in_maps = []
    for i in range(NCORES):
        in_maps.append({
            "x1s": _chunk_img(np.asarray(x1[i], np.float32).reshape(C, N)),
            "x2s": _chunk_img(np.asarray(x2[i], np.float32).reshape(C, N)),
            "xs": _chunk_img(np.asarray(x[i], np.float32).reshape(C, N)),
            "w1t": w1t, "w2t": w2t, "vecs": vecs,
        })
    res = run_bass_kernel_spmd(nc, in_maps, list(range(NCORES)), **run_kw)
    out = np.empty((B, C, H, W), dtype=np.float32)
    for i in range(NCORES):
        o = res.results[i]["out"]  # [128, 4096]
        out[i] = o.reshape(128, KC, N).transpose(1, 0, 2).reshape(C, H, W)
    if run_kw:
        kernel.last_results = res
    return out
